# revision 34
# baseline (speedup 1.0000x reference)
"""Trainium2 Bass kernel for the DNA/protein PWM-scan block.

Math (per batch row, see reference):
    score_f = valid_conv(DNA, PWM)   # 12 taps x 4 channels
    score_r = valid_conv(DNA, PWMrc)
    m       = max(score_f, score_r)
    k_relu  = (m > 0) * exp(lam * (m - 10))
    out     = zero_pad(k_relu, L+1) * concen

Kernel strategy (8 NeuronCores, one batch row per core), v2:
  All streaming data is fp16 (validated rel err ~8e-4 vs the 2e-3 gate;
  scores accumulate in fp32 PSUM).  Flatten DNA row to x[4l+c].  A block
  of 64 consecutive positions starting at flat offset 256j spans 300 flat
  elements, so with x in 128-element flat columns (xt col = 128n + P for
  partition P, block n):

      P[0:64,  (jj,P)] = fwd scores of positions 64*(4g+jj)+m
      P[64:128,(jj,P)] = rev scores
      P[:, (jj,P)]     = sum_t W_t.T @ xt[:, (2(4g+jj)+t)*128 + P]

  with three [128,128] band weight tiles W0/W1/W2 (cols 0-63 fwd,
  64-127 rev).  Pipeline per super-tile: DMA natural-layout fp16 DNA ->
  PE transpose (fp16: 1 cyc/row) to flat columns -> 3 accumulating PE
  matmuls per group -> one ACT exp (psum fp32 -> fp16, with the score
  threshold folded in via exp monotonicity: s>0 <=> e^s' > thr') ->
  DVE max over strands + threshold-mask -> PE transpose back (fp16) ->
  DVE multiply by concen (prescaled by e^C on host against fp16
  overflow) -> DMA out fp16, host upcasts.
"""

import os
from contextlib import ExitStack

import numpy as np

import concourse.bass as bass
import concourse.tile as tile
from concourse import mybir
from concourse.bass_utils import run_bass_kernel_spmd
from concourse.tile import ScopedClock

F32 = mybir.dt.float32
F16 = mybir.dt.float16


def _drain_and_barrier_split(self, tick_clock, wait_clock):
    """TileContext kernel-tail drain, with sem waits split one per Drain.

    The pinned walrus build rejects TPB_CTRL instructions carrying more
    than one sync-wait command ("Too many sync wait commands"), and the
    stock tail drain accumulates one wait per outstanding semaphore.
    Emitting a chain of single-wait drains is semantically identical
    (waits are conjunctive and the SP queue is sequential).
    """
    nc = self.nc
    drain_inst = nc.sync.drain()
    wait_clock.add_sem_waits(
        drain_inst.ins, ScopedClock({None: tick_clock.global_clock})
    )
    ins = drain_inst.ins
    waits = list(ins.sync_info.on_wait)
    if len(waits) > 1:
        si = ins.sync_info
        si.on_wait = waits[:1]
        ins.sync_info = si
        for wi in waits[1:]:
            d2 = nc.sync.drain()
            d2.ins.sync_info = mybir.SyncInfo(on_wait=[wi], on_update=[])
    nc.all_engine_barrier()
    popped = nc._tile_sem_poison_stack.pop()
    assert popped is self._sem_poison
    nc.clear_and_free_semaphores(list(self.sems.allocated().values()))
    nc.all_engine_barrier()


tile.TileContext._drain_and_barrier = _drain_and_barrier_split

_orig_add_instruction = tile.TileContext._add_instruction
_wsplit_counter = [0]


def _add_instruction_split_waits(self, inst):
    """Cap every committed instruction at one sync wait.

    Same walrus limitation as the drain: engine instructions (e.g. the
    S3_LW half of Matmult) reject >1 sync-wait command. Excess waits are
    peeled onto no-op carriers emitted just before, on the same engine
    queue, which is semantically equivalent for conjunctive waits.
    """
    si = getattr(inst, "sync_info", None)
    if si is not None and si.on_wait and len(si.on_wait) > 1:
        waits = list(si.on_wait)
        for wi in waits[:-1]:
            _wsplit_counter[0] += 1
            nop = mybir.InstNoOp(
                name=f"wsplit-{_wsplit_counter[0]}",
                sync_info=mybir.SyncInfo(on_wait=[wi], on_update=[]),
                bass_nofuse=True,
                engine=inst.engine,
            )
            _orig_add_instruction(self, nop)
        si.on_wait = waits[-1:]
        inst.sync_info = si
    _orig_add_instruction(self, inst)


tile.TileContext._add_instruction = _add_instruction_split_waits

# ---------------------------------------------------------------- geometry

B = 8
L = 500_000
STEP = 12
MAX_S = 10.0
NV = L - STEP + 1          # 499_989 valid conv outputs
LO = L + 1                 # padded output length
N4 = 4 * L                 # flattened DNA length per row
C_RESCALE = 7.0            # exp output prescale: e^{lam(s-10)-C}, cc *= e^C


def _geometry(n4, nv, c4):
    """Super-tile bases (flat-element offsets) covering [0, nv) positions."""
    assert c4 % 1024 == 0
    sp = 32 * c4                    # positions per super-tile
    assert nv >= sp
    n_full = nv // sp
    bases = [t * 128 * c4 for t in range(n_full)]
    if n_full * sp < nv:
        bases.append(4 * (nv - sp))  # overlapping final tile, ends at nv
    return bases, sp


def _band_weights3(PWM, PWMrc):
    """Three [128,128] stationary tiles; cols 0-63 fwd / 64-127 rev.

    W_t[k, m] = w[128*t + k - 4*m] for the 48-tap stride-4 band, so that
    sum_t W_t.T @ x[256j + 128t : +128] gives scores for positions
    64j..64j+63 of both strands.
    """
    wf = np.asarray(PWM, np.float32).reshape(-1)
    wr = np.asarray(PWMrc, np.float32).reshape(-1)
    W = np.zeros((3, 128, 128), np.float32)
    for m in range(64):
        for j in range(48):
            p = 4 * m + j
            t, k = divmod(p, 128)
            W[t, k, m] = wf[j]
            W[t, k, 64 + m] = wr[j]
    return np.concatenate([W[0], W[1], W[2]], axis=1)  # [128, 384]


def _dap(t, offset, pattern):
    return bass.AP(tensor=t, offset=offset, ap=[list(p) for p in pattern])


def build_nc(n4=N4, nv=NV, lo=LO, c4=4096, iters=1,
             nat_bufs=3, xt_bufs=3, ew_bufs=4, io_bufs=3, cc_bufs=5,
             xt_dve_phase=1, ps_t_bufs=3, ps_b_bufs=2, ps_o_bufs=2):
    """Build the single-core Bass program (SPMD across 8 cores)."""
    nc = bass.Bass("TRN2", target_bir_lowering=False, debug=False)

    dna = nc.dram_tensor("dna", [n4], F16, kind="ExternalInput")
    conc = nc.dram_tensor("conc", [lo], F16, kind="ExternalInput")
    w_d = nc.dram_tensor("wband", [128, 384], F16, kind="ExternalInput")
    lam_d = nc.dram_tensor("lam", [1, 1], F32, kind="ExternalInput")
    out = nc.dram_tensor("out", [lo], F16, kind="ExternalOutput")

    bases, sp = _geometry(n4, nv, c4)
    cp = c4 // 4                # positions per partition chunk
    nb = c4 // 128              # full x-column blocks per partition
    tin = nb + 1                # +1 halo block
    ngrp = cp // 256            # matmul groups (4 output blocks each)
    assert ngrp % 2 == 0

    with ExitStack() as ctx:
        tc = ctx.enter_context(tile.TileContext(nc))
        consts = ctx.enter_context(tc.tile_pool(name="consts", bufs=1))
        natp = ctx.enter_context(tc.tile_pool(name="nat", bufs=nat_bufs))
        xtp = ctx.enter_context(tc.tile_pool(name="xt", bufs=xt_bufs))
        ewp = ctx.enter_context(tc.tile_pool(name="ew", bufs=ew_bufs))
        iop = ctx.enter_context(tc.tile_pool(name="io", bufs=io_bufs))
        ccp = ctx.enter_context(tc.tile_pool(name="ccp", bufs=cc_bufs))
        natL = ctx.enter_context(tc.tile_pool(name="natL", bufs=2))
        ps_t = ctx.enter_context(tc.tile_pool(name="ps_t", bufs=ps_t_bufs, space="PSUM"))
        ps_b = ctx.enter_context(tc.tile_pool(name="ps_b", bufs=ps_b_bufs, space="PSUM"))
        ps_o = ctx.enter_context(tc.tile_pool(name="ps_o", bufs=ps_o_bufs, space="PSUM"))

        w_sb = consts.tile([128, 384], F16)
        nc.sync.dma_start(w_sb, w_d.ap())
        lam_sb = consts.tile([128, 1], F32)
        nc.sync.dma_start(lam_sb, _dap(lam_d, 0, [[0, 128], [1, 1]]))
        # identity for PE transposes, synthesized on Pool (no DMA: a DMA'd
        # identity round-robins behind the streaming loads and stalls PE).
        id_sb = consts.tile([128, 128], F16)
        nc.gpsimd.memset(id_sb, 1.0)
        nc.gpsimd.affine_select(
            id_sb, id_sb, [[1, 128]], mybir.AluOpType.is_equal, 0.0,
            base=0, channel_multiplier=-1,
        )
        # exp argument affine: lam*s - 10*lam - C.  The score threshold is
        # free: e^{lam*s-10lam-C} for s<=0 is below the smallest fp16
        # subnormal (requires 10*lam + C > 16.6; host asserts), so the
        # fp16 exp output IS the masked k_relu value.
        nlamC_sb = consts.tile([128, 1], F32)
        nc.vector.tensor_scalar_mul(nlamC_sb, lam_sb, -MAX_S)
        nc.vector.tensor_scalar_add(nlamC_sb, nlamC_sb, -C_RESCALE)

        # dedicated double-buffer for the tail-clamped final super-tile:
        # its zero-padding never changes, so memset once here instead of
        # on the per-iteration critical path.
        nt_lasts = []
        for b_t in bases:
            if min(c4 + 128, n4 - (b_t + 127 * c4)) < c4 + 128:
                w_l = n4 - (b_t + 127 * c4)
                for _ in range(2):
                    ntl = natL.tile([128, c4 + 128], F16, tag="ntl")
                    nc.vector.memset(ntl[:, w_l : c4 + 128].bitcast(F32), 0.0)
                    nt_lasts.append(ntl)

        # out-stores are deferred past the NEXT tile's load issues: the
        # DMA engines drain roughly in issue order, and a store that waits
        # on the tail of a tile's compute at the head of the line would
        # stall the following loads.
        pending_stores = []

        def _flush_stores():
            while pending_stores:
                dst, src = pending_stores.pop(0)
                nc.gpsimd.dma_start(dst, src)

        # software-pipelined load issue, two tiles ahead of compute, so
        # the config -> descgen -> trigger chain hides under wire time.
        tiles = [b_t for _ in range(iters) for b_t in bases]
        loaded = {}

        def _issue_loads(k):
            if k >= len(tiles):
                return
            b_t = tiles[k]
            w_last = min(c4 + 128, n4 - (b_t + 127 * c4))
            if w_last == c4 + 128:
                nt = natp.tile([128, c4 + 128], F16, tag="nt")
                # two half-loads so the first transpose group can start
                # as soon as the leading columns land.
                wh = (c4 + 128) // 2
                nc.sync.dma_start(
                    nt[:, 0:wh], _dap(dna, b_t, [[c4, 128], [1, wh]])
                )
                nc.sync.dma_start(
                    nt[:, wh : c4 + 128],
                    _dap(dna, b_t + wh, [[c4, 128], [1, c4 + 128 - wh]]),
                )
            else:
                # final partition would read past the end of the row:
                # clamp its DMA (the out-of-range tail is pre-zeroed in
                # nt_lasts; it only meets zero rows of the band weights,
                # but NaNs would still poison psum).
                nt = nt_lasts[(k // len(bases)) % 2]
                nc.sync.dma_start(
                    nt[0:127, :], _dap(dna, b_t, [[c4, 127], [1, c4 + 128]])
                )
                nc.sync.dma_start(
                    nt[127:128, 0:w_last],
                    _dap(dna, b_t + 127 * c4, [[1, w_last]]),
                )
            cc = ccp.tile([128, cp], F16, tag="cc")
            nc.sync.dma_start(cc, _dap(conc, p_t_of(b_t), [[cp, 128], [1, cp]]))
            loaded[k] = (nt, cc)

        def p_t_of(b_t):
            return b_t // 4

        _issue_loads(0)
        _issue_loads(1)
        for k in range(len(tiles)):
            b_t = tiles[k]
            if True:
                p_t = b_t // 4
                nt, cc = loaded.pop(k)
                ot = iop.tile([128, cp], F16, tag="ot")
                _issue_loads(k + 2)
                _flush_stores()

                # xt is padded by 128 cols: the strided matmul views of the
                # last group rearrange over [base, base+1024) before
                # sub-slicing back inside the written region.
                xt = xtp.tile([128, c4 + 256], F16, tag="xt")
                grp_done = [False] * ngrp
                kts = [None] * (ngrp // 2)

                def _emit_group(g):
                    pb = ps_b.tile([128, 512], F32, tag="pb")
                    for t in range(3):
                        base = 1024 * g + 128 * t
                        mv = xt[:, base : base + 1024].rearrange(
                            "a (j r) -> a j r", r=256
                        )[:, :, 0:128]
                        nc.tensor.matmul(
                            pb, w_sb[:, 128 * t : 128 * t + 128], mv,
                            start=(t == 0), stop=(t == 2),
                        )
                    ex = ewp.tile([128, 512], F16, tag="ex")
                    nc.scalar.activation(
                        ex, pb, mybir.ActivationFunctionType.Exp,
                        bias=nlamC_sb, scale=lam_sb,
                    )
                    # strand pairing: the rev half lives at partitions
                    # 64-127; TensorTensor SBUF inputs must share a start
                    # partition, so stage it down to 0-63 with a remapping
                    # copy, then max into the kt pair tile (output remap
                    # is allowed).
                    er = ewp.tile([64, 512], F16, tag="er")
                    nc.vector.tensor_copy(er, ex[64:128, :])
                    q, h = divmod(g, 2)
                    if h == 0:
                        ktt = ewp.tile([128, 512], F16, tag="kt")
                        kts[q] = ktt
                    nc.vector.tensor_tensor(
                        kts[q][64 * h : 64 * h + 64, :],
                        ex[0:64, :], er, mybir.AluOpType.max,
                    )
                    grp_done[g] = True
                    if h == 1:
                        _emit_pair(q)

                def _emit_pair(q):
                    kt = kts[q]
                    po = ps_o.tile([128, 512], F16, tag="po")
                    for b in range(4):
                        nc.tensor.transpose(
                            po[:, 128 * b : 128 * b + 128],
                            kt[:, 128 * b : 128 * b + 128],
                            id_sb,
                        )
                    po_v = po.rearrange("a (b h x) -> a b h x", b=4, h=2)
                    ot_v = ot[:, 512 * q : 512 * q + 512].rearrange(
                        "a (h b x) -> a b h x", h=2, b=4
                    )
                    cc_v = cc[:, 512 * q : 512 * q + 512].rearrange(
                        "a (h b x) -> a b h x", h=2, b=4
                    )
                    nc.vector.tensor_tensor(
                        ot_v, po_v, cc_v, mybir.AluOpType.mult
                    )
                    pending_stores.append((
                        _dap(out, p_t + 512 * q, [[cp, 128], [1, 512]]),
                        ot[:, 512 * q : 512 * q + 512],
                    ))

                done = 0
                while done < tin:
                    nblk = min(8, tin - done)
                    pt = ps_t.tile([128, 1024], F16, tag="pt")
                    for i in range(nblk):
                        bcol = (done + i) * 128
                        nc.tensor.transpose(
                            pt[:, i * 128 : (i + 1) * 128],
                            nt[:, bcol : bcol + 128],
                            id_sb,
                        )
                    # psum->sbuf staging: DVE copies run ~2.3x faster than
                    # ACT copies (16-bit 2x mode); ACT takes one batch per
                    # super-tile to keep DVE off the critical path.  GPSIMD
                    # cannot read PSUM on this toolchain.
                    if (done // 8) == xt_dve_phase:
                        nc.scalar.activation(
                            xt[:, done * 128 : (done + nblk) * 128],
                            pt[:, : nblk * 128],
                            mybir.ActivationFunctionType.Copy,
                        )
                    else:
                        nc.vector.tensor_copy(
                            xt[:, done * 128 : (done + nblk) * 128],
                            pt[:, : nblk * 128],
                        )
                    done += nblk
                    # emit each group as soon as the x-columns it reads
                    # (blocks 0 .. 8g+8 inclusive) are staged in SBUF.
                    for g in range(ngrp):
                        if not grp_done[g] and done >= 8 * g + 9:
                            _emit_group(g)
                for g in range(ngrp):
                    if not grp_done[g]:
                        _emit_group(g)
        _flush_stores()
    return nc


# ------------------------------------------------------------------ driver

_CACHE = {}

BEST_CFG = dict(
    c4=4096,
    nat_bufs=3,
    xt_bufs=3,
    ew_bufs=4,
    io_bufs=3,
)


def _get_nc(key, **kw):
    if key not in _CACHE:
        _CACHE[key] = build_nc(**kw)
    return _CACHE[key]


def make_in_maps(DNA, concen, PWM, PWMrc, lam):
    Wb = _band_weights3(PWM, PWMrc).astype(np.float16)
    lam_v = np.asarray(lam, np.float32).reshape(1, 1)
    # mask-free threshold condition: e^{-10 lam - C} must underflow fp16
    # (see build_nc); holds for the reference's fixed lam=1.166.
    assert 10.0 * float(lam_v[0, 0]) + C_RESCALE > 16.9, (
        "lam too small for the threshold-free fp16 exp trick"
    )
    dna_rows = np.ascontiguousarray(
        np.asarray(DNA, np.float32).reshape(B, N4).astype(np.float16)
    )
    conc_rows = np.ascontiguousarray(
        (np.asarray(concen, np.float32).reshape(B, LO)
         * np.float32(np.exp(C_RESCALE))).astype(np.float16)
    )
    return [
        {
            "dna": dna_rows[r],
            "conc": conc_rows[r],
            "wband": Wb,
            "lam": lam_v,
        }
        for r in range(B)
    ]


LAST_RESULTS = None


def kernel(DNA, concen, PWM, PWMrc, lam):
    global LAST_RESULTS
    nc = _get_nc("main", **BEST_CFG)
    in_maps = make_in_maps(DNA, concen, PWM, PWMrc, lam)
    res = run_bass_kernel_spmd(nc, in_maps, core_ids=list(range(B)))
    LAST_RESULTS = res
    rows = [res.results[r]["out"].astype(np.float32) for r in range(B)]
    out = np.stack(rows, axis=0).reshape(B, LO, 1, 1)
    return out


# revision 36
# speedup vs baseline: 1.1740x; 1.1740x over previous
"""Trainium2 Bass kernel for the DNA/protein PWM-scan block.

Math (per batch row, see reference):
    score_f = valid_conv(DNA, PWM)   # 12 taps x 4 channels
    score_r = valid_conv(DNA, PWMrc)
    m       = max(score_f, score_r)
    k_relu  = (m > 0) * exp(lam * (m - 10))
    out     = zero_pad(k_relu, L+1) * concen

Kernel strategy (8 NeuronCores, one batch row per core), v2:
  All streaming data is fp16 (validated rel err ~8e-4 vs the 2e-3 gate;
  scores accumulate in fp32 PSUM).  Flatten DNA row to x[4l+c].  A block
  of 64 consecutive positions starting at flat offset 256j spans 300 flat
  elements, so with x in 128-element flat columns (xt col = 128n + P for
  partition P, block n):

      P[0:64,  (jj,P)] = fwd scores of positions 64*(4g+jj)+m
      P[64:128,(jj,P)] = rev scores
      P[:, (jj,P)]     = sum_t W_t.T @ xt[:, (2(4g+jj)+t)*128 + P]

  with three [128,128] band weight tiles W0/W1/W2 (cols 0-63 fwd,
  64-127 rev).  Pipeline per super-tile: DMA natural-layout fp16 DNA ->
  PE transpose (fp16: 1 cyc/row) to flat columns -> 3 accumulating PE
  matmuls per group -> one ACT exp (psum fp32 -> fp16, with the score
  threshold folded in via exp monotonicity: s>0 <=> e^s' > thr') ->
  DVE max over strands + threshold-mask -> PE transpose back (fp16) ->
  DVE multiply by concen (prescaled by e^C on host against fp16
  overflow) -> DMA out fp16, host upcasts.
"""

import os
from contextlib import ExitStack

import numpy as np

import concourse.bass as bass
import concourse.tile as tile
from concourse import mybir
from concourse.bass_utils import run_bass_kernel_spmd
from concourse.tile import ScopedClock

F32 = mybir.dt.float32
F16 = mybir.dt.float16


def _drain_and_barrier_split(self, tick_clock, wait_clock):
    """TileContext kernel-tail drain, with sem waits split one per Drain.

    The pinned walrus build rejects TPB_CTRL instructions carrying more
    than one sync-wait command ("Too many sync wait commands"), and the
    stock tail drain accumulates one wait per outstanding semaphore.
    Emitting a chain of single-wait drains is semantically identical
    (waits are conjunctive and the SP queue is sequential).
    """
    nc = self.nc
    drain_inst = nc.sync.drain()
    wait_clock.add_sem_waits(
        drain_inst.ins, ScopedClock({None: tick_clock.global_clock})
    )
    ins = drain_inst.ins
    waits = list(ins.sync_info.on_wait)
    if len(waits) > 1:
        si = ins.sync_info
        si.on_wait = waits[:1]
        ins.sync_info = si
        for wi in waits[1:]:
            d2 = nc.sync.drain()
            d2.ins.sync_info = mybir.SyncInfo(on_wait=[wi], on_update=[])
    nc.all_engine_barrier()
    popped = nc._tile_sem_poison_stack.pop()
    assert popped is self._sem_poison
    nc.clear_and_free_semaphores(list(self.sems.allocated().values()))
    nc.all_engine_barrier()


tile.TileContext._drain_and_barrier = _drain_and_barrier_split

_orig_add_instruction = tile.TileContext._add_instruction
_wsplit_counter = [0]


def _add_instruction_split_waits(self, inst):
    """Cap every committed instruction at one sync wait.

    Same walrus limitation as the drain: engine instructions (e.g. the
    S3_LW half of Matmult) reject >1 sync-wait command. Excess waits are
    peeled onto no-op carriers emitted just before, on the same engine
    queue, which is semantically equivalent for conjunctive waits.
    """
    si = getattr(inst, "sync_info", None)
    if si is not None and si.on_wait and len(si.on_wait) > 1:
        waits = list(si.on_wait)
        for wi in waits[:-1]:
            _wsplit_counter[0] += 1
            nop = mybir.InstNoOp(
                name=f"wsplit-{_wsplit_counter[0]}",
                sync_info=mybir.SyncInfo(on_wait=[wi], on_update=[]),
                bass_nofuse=True,
                engine=inst.engine,
            )
            _orig_add_instruction(self, nop)
        si.on_wait = waits[-1:]
        inst.sync_info = si
    _orig_add_instruction(self, inst)


tile.TileContext._add_instruction = _add_instruction_split_waits

# ---------------------------------------------------------------- geometry

B = 8
L = 500_000
STEP = 12
MAX_S = 10.0
NV = L - STEP + 1          # 499_989 valid conv outputs
LO = L + 1                 # padded output length
N4 = 4 * L                 # flattened DNA length per row
C_RESCALE = 7.0            # exp output prescale: e^{lam(s-10)-C}, cc *= e^C


def _geometry(n4, nv, c4):
    """Super-tile bases (flat-element offsets) covering [0, nv) positions."""
    assert c4 % 1024 == 0
    sp = 32 * c4                    # positions per super-tile
    assert nv >= sp
    n_full = nv // sp
    bases = [t * 128 * c4 for t in range(n_full)]
    if n_full * sp < nv:
        bases.append(4 * (nv - sp))  # overlapping final tile, ends at nv
    return bases, sp


def _band_weights3(PWM, PWMrc):
    """Three [128,128] stationary tiles; cols 0-63 fwd / 64-127 rev.

    W_t[k, m] = w[128*t + k - 4*m] for the 48-tap stride-4 band, so that
    sum_t W_t.T @ x[256j + 128t : +128] gives scores for positions
    64j..64j+63 of both strands.
    """
    wf = np.asarray(PWM, np.float32).reshape(-1)
    wr = np.asarray(PWMrc, np.float32).reshape(-1)
    W = np.zeros((3, 128, 128), np.float32)
    for m in range(64):
        for j in range(48):
            p = 4 * m + j
            t, k = divmod(p, 128)
            W[t, k, m] = wf[j]
            W[t, k, 64 + m] = wr[j]
    return np.concatenate([W[0], W[1], W[2]], axis=1)  # [128, 384]


def _dap(t, offset, pattern):
    return bass.AP(tensor=t, offset=offset, ap=[list(p) for p in pattern])


def build_nc(n4=N4, nv=NV, lo=LO, c4=4096, iters=1,
             nat_bufs=3, xt_bufs=3, ew_bufs=4, io_bufs=3, cc_bufs=5,
             xt_dve_phase=1, ps_t_bufs=3, ps_b_bufs=2, ps_o_bufs=2,
             ahead=2):
    """Build the single-core Bass program (SPMD across 8 cores)."""
    nc = bass.Bass("TRN2", target_bir_lowering=False, debug=False)

    dna = nc.dram_tensor("dna", [n4], F16, kind="ExternalInput")
    conc = nc.dram_tensor("conc", [lo], F16, kind="ExternalInput")
    w_d = nc.dram_tensor("wband", [128, 384], F16, kind="ExternalInput")
    lam_d = nc.dram_tensor("lam", [1, 1], F32, kind="ExternalInput")
    out = nc.dram_tensor("out", [lo], F16, kind="ExternalOutput")

    bases, sp = _geometry(n4, nv, c4)
    cp = c4 // 4                # positions per partition chunk
    nb = c4 // 128              # full x-column blocks per partition
    tin = nb + 1                # +1 halo block
    ngrp = cp // 256            # matmul groups (4 output blocks each)
    assert ngrp % 2 == 0

    with ExitStack() as ctx:
        tc = ctx.enter_context(tile.TileContext(nc))
        consts = ctx.enter_context(tc.tile_pool(name="consts", bufs=1))
        natp = ctx.enter_context(tc.tile_pool(name="nat", bufs=nat_bufs))
        xtp = ctx.enter_context(tc.tile_pool(name="xt", bufs=xt_bufs))
        ewp = ctx.enter_context(tc.tile_pool(name="ew", bufs=ew_bufs))
        iop = ctx.enter_context(tc.tile_pool(name="io", bufs=io_bufs))
        ccp = ctx.enter_context(tc.tile_pool(name="ccp", bufs=cc_bufs))
        natL = ctx.enter_context(tc.tile_pool(name="natL", bufs=2))
        ps_t = ctx.enter_context(tc.tile_pool(name="ps_t", bufs=ps_t_bufs, space="PSUM"))
        ps_b = ctx.enter_context(tc.tile_pool(name="ps_b", bufs=ps_b_bufs, space="PSUM"))
        ps_o = ctx.enter_context(tc.tile_pool(name="ps_o", bufs=ps_o_bufs, space="PSUM"))

        w_sb = consts.tile([128, 384], F16)
        nc.sync.dma_start(w_sb, w_d.ap())
        lam_sb = consts.tile([128, 1], F32)
        nc.sync.dma_start(lam_sb, _dap(lam_d, 0, [[0, 128], [1, 1]]))
        # identity for PE transposes, synthesized on Pool (no DMA: a DMA'd
        # identity round-robins behind the streaming loads and stalls PE).
        id_sb = consts.tile([128, 128], F16)
        nc.gpsimd.memset(id_sb, 1.0)
        nc.gpsimd.affine_select(
            id_sb, id_sb, [[1, 128]], mybir.AluOpType.is_equal, 0.0,
            base=0, channel_multiplier=-1,
        )
        # exp argument affine: lam*s - 10*lam - C.  The score threshold is
        # free: e^{lam*s-10lam-C} for s<=0 is below the smallest fp16
        # subnormal (requires 10*lam + C > 16.6; host asserts), so the
        # fp16 exp output IS the masked k_relu value.
        nlamC_sb = consts.tile([128, 1], F32)
        nc.vector.tensor_scalar_mul(nlamC_sb, lam_sb, -MAX_S)
        nc.vector.tensor_scalar_add(nlamC_sb, nlamC_sb, -C_RESCALE)

        # dedicated double-buffer for the tail-clamped final super-tile:
        # its zero-padding never changes, so memset once here instead of
        # on the per-iteration critical path.
        nt_lasts = []
        for b_t in bases:
            if min(c4 + 128, n4 - (b_t + 127 * c4)) < c4 + 128:
                w_l = n4 - (b_t + 127 * c4)
                for _ in range(2):
                    ntl = natL.tile([128, c4 + 128], F16, tag="ntl")
                    nc.vector.memset(ntl[:, w_l : c4 + 128].bitcast(F32), 0.0)
                    nt_lasts.append(ntl)

        # out-stores are deferred past the NEXT tile's load issues: the
        # DMA engines drain roughly in issue order, and a store that waits
        # on the tail of a tile's compute at the head of the line would
        # stall the following loads.
        pending_stores = []

        def _flush_stores():
            while pending_stores:
                dst, src = pending_stores.pop(0)
                nc.gpsimd.dma_start(dst, src)

        # software-pipelined load issue, two tiles ahead of compute, so
        # the config -> descgen -> trigger chain hides under wire time.
        tiles = [b_t for _ in range(iters) for b_t in bases]
        loaded = {}

        def _issue_loads(k):
            if k >= len(tiles):
                return
            b_t = tiles[k]
            w_last = min(c4 + 128, n4 - (b_t + 127 * c4))
            if w_last == c4 + 128:
                nt = natp.tile([128, c4 + 128], F16, tag="nt")
                # two half-loads so the first transpose group can start
                # as soon as the leading columns land.
                wh = (c4 + 128) // 2
                nc.sync.dma_start(
                    nt[:, 0:wh], _dap(dna, b_t, [[c4, 128], [1, wh]])
                )
                nc.sync.dma_start(
                    nt[:, wh : c4 + 128],
                    _dap(dna, b_t + wh, [[c4, 128], [1, c4 + 128 - wh]]),
                )
            else:
                # final partition would read past the end of the row:
                # clamp its DMA (the out-of-range tail is pre-zeroed in
                # nt_lasts; it only meets zero rows of the band weights,
                # but NaNs would still poison psum).
                nt = nt_lasts[(k // len(bases)) % 2]
                nc.sync.dma_start(
                    nt[0:127, :], _dap(dna, b_t, [[c4, 127], [1, c4 + 128]])
                )
                nc.sync.dma_start(
                    nt[127:128, 0:w_last],
                    _dap(dna, b_t + 127 * c4, [[1, w_last]]),
                )
            cc = ccp.tile([128, cp], F16, tag="cc")
            nc.sync.dma_start(cc, _dap(conc, p_t_of(b_t), [[cp, 128], [1, cp]]))
            loaded[k] = (nt, cc)

        def p_t_of(b_t):
            return b_t // 4

        for j in range(ahead):
            _issue_loads(j)
        for k in range(len(tiles)):
            b_t = tiles[k]
            if True:
                p_t = b_t // 4
                if ahead == 0:
                    _issue_loads(k)
                nt, cc = loaded.pop(k)
                ot = iop.tile([128, cp], F16, tag="ot")
                if ahead > 0:
                    _issue_loads(k + ahead)
                _flush_stores()

                # xt is padded by 128 cols: the strided matmul views of the
                # last group rearrange over [base, base+1024) before
                # sub-slicing back inside the written region.
                xt = xtp.tile([128, c4 + 256], F16, tag="xt")
                grp_done = [False] * ngrp
                kts = [None] * (ngrp // 2)

                def _emit_group(g):
                    pb = ps_b.tile([128, 512], F32, tag="pb")
                    for t in range(3):
                        base = 1024 * g + 128 * t
                        mv = xt[:, base : base + 1024].rearrange(
                            "a (j r) -> a j r", r=256
                        )[:, :, 0:128]
                        nc.tensor.matmul(
                            pb, w_sb[:, 128 * t : 128 * t + 128], mv,
                            start=(t == 0), stop=(t == 2),
                        )
                    ex = ewp.tile([128, 512], F16, tag="ex")
                    nc.scalar.activation(
                        ex, pb, mybir.ActivationFunctionType.Exp,
                        bias=nlamC_sb, scale=lam_sb,
                    )
                    # strand pairing: the rev half lives at partitions
                    # 64-127; TensorTensor SBUF inputs must share a start
                    # partition, so stage it down to 0-63 with a remapping
                    # copy, then max into the kt pair tile (output remap
                    # is allowed).
                    er = ewp.tile([64, 512], F16, tag="er")
                    nc.vector.tensor_copy(er, ex[64:128, :])
                    q, h = divmod(g, 2)
                    if h == 0:
                        ktt = ewp.tile([128, 512], F16, tag="kt")
                        kts[q] = ktt
                    nc.vector.tensor_tensor(
                        kts[q][64 * h : 64 * h + 64, :],
                        ex[0:64, :], er, mybir.AluOpType.max,
                    )
                    grp_done[g] = True
                    if h == 1:
                        _emit_pair(q)

                def _emit_pair(q):
                    kt = kts[q]
                    po = ps_o.tile([128, 512], F16, tag="po")
                    for b in range(4):
                        nc.tensor.transpose(
                            po[:, 128 * b : 128 * b + 128],
                            kt[:, 128 * b : 128 * b + 128],
                            id_sb,
                        )
                    po_v = po.rearrange("a (b h x) -> a b h x", b=4, h=2)
                    ot_v = ot[:, 512 * q : 512 * q + 512].rearrange(
                        "a (h b x) -> a b h x", h=2, b=4
                    )
                    cc_v = cc[:, 512 * q : 512 * q + 512].rearrange(
                        "a (h b x) -> a b h x", h=2, b=4
                    )
                    nc.vector.tensor_tensor(
                        ot_v, po_v, cc_v, mybir.AluOpType.mult
                    )
                    pending_stores.append((
                        _dap(out, p_t + 512 * q, [[cp, 128], [1, 512]]),
                        ot[:, 512 * q : 512 * q + 512],
                    ))

                done = 0
                while done < tin:
                    nblk = min(8, tin - done)
                    pt = ps_t.tile([128, 1024], F16, tag="pt")
                    for i in range(nblk):
                        bcol = (done + i) * 128
                        nc.tensor.transpose(
                            pt[:, i * 128 : (i + 1) * 128],
                            nt[:, bcol : bcol + 128],
                            id_sb,
                        )
                    # psum->sbuf staging: DVE copies run ~2.3x faster than
                    # ACT copies (16-bit 2x mode); ACT takes one batch per
                    # super-tile to keep DVE off the critical path.  GPSIMD
                    # cannot read PSUM on this toolchain.
                    if (done // 8) == xt_dve_phase:
                        nc.scalar.activation(
                            xt[:, done * 128 : (done + nblk) * 128],
                            pt[:, : nblk * 128],
                            mybir.ActivationFunctionType.Copy,
                        )
                    else:
                        nc.vector.tensor_copy(
                            xt[:, done * 128 : (done + nblk) * 128],
                            pt[:, : nblk * 128],
                        )
                    done += nblk
                    # emit each group as soon as the x-columns it reads
                    # (blocks 0 .. 8g+8 inclusive) are staged in SBUF.
                    for g in range(ngrp):
                        if not grp_done[g] and done >= 8 * g + 9:
                            _emit_group(g)
                for g in range(ngrp):
                    if not grp_done[g]:
                        _emit_group(g)
        _flush_stores()
    return nc


# ------------------------------------------------------------------ driver

_CACHE = {}

BEST_CFG = dict(
    c4=4096,
    nat_bufs=3,
    xt_bufs=3,
    ew_bufs=4,
    io_bufs=3,
)


def _get_nc(key, **kw):
    if key not in _CACHE:
        _CACHE[key] = build_nc(**kw)
    return _CACHE[key]


def make_in_maps(DNA, concen, PWM, PWMrc, lam):
    Wb = _band_weights3(PWM, PWMrc).astype(np.float16)
    lam_v = np.asarray(lam, np.float32).reshape(1, 1)
    # mask-free threshold condition: e^{-10 lam - C} must underflow fp16
    # (see build_nc); holds for the reference's fixed lam=1.166.
    assert 10.0 * float(lam_v[0, 0]) + C_RESCALE > 16.9, (
        "lam too small for the threshold-free fp16 exp trick"
    )
    dna_rows = np.ascontiguousarray(
        np.asarray(DNA, np.float32).reshape(B, N4).astype(np.float16)
    )
    conc_rows = np.ascontiguousarray(
        (np.asarray(concen, np.float32).reshape(B, LO)
         * np.float32(np.exp(C_RESCALE))).astype(np.float16)
    )
    return [
        {
            "dna": dna_rows[r],
            "conc": conc_rows[r],
            "wband": Wb,
            "lam": lam_v,
        }
        for r in range(B)
    ]


LAST_RESULTS = None


def kernel(DNA, concen, PWM, PWMrc, lam):
    global LAST_RESULTS
    nc = _get_nc("main", **BEST_CFG)
    in_maps = make_in_maps(DNA, concen, PWM, PWMrc, lam)
    res = run_bass_kernel_spmd(nc, in_maps, core_ids=list(range(B)))
    LAST_RESULTS = res
    rows = [res.results[r]["out"].astype(np.float32) for r in range(B)]
    out = np.stack(rows, axis=0).reshape(B, LO, 1, 1)
    return out
